# revision 36
# baseline (speedup 1.0000x reference)
"""Channel-attention (CAM) Trainium2 kernel — fp8 DoubleRow mm2 version.

Reference computation (per batch b of 16):
    q   = x[b].reshape(C, HW)                  # C=512, HW=4096
    sim = q @ q.T                              # [C, C], symmetric
    attn = softmax(max(sim) - sim, axis=-1)    # == exp(min_r - sim) / Z_r
    out[b] = gamma * attn @ q + x[b]

Sharding: data-parallel over batch across 8 NeuronCores (2 batches/core).
kernel() takes full inputs, shards internally, returns the full output.

Design notes (per-core, derived from HW trace analysis of the f32r
baseline — PE sustains ~1.8-2.3GHz, 1 cyc/row f32r, transposes 1.5x):
  - x is DMA'd straight into f32r qr tiles (f32<->f32r bitcast on both
    sides of the DMA: f32r is an fp32-storage PE read mode, and the DMA
    itself satisfies the BIR verifier's "rounded to f32r" producer rule).
    No DVE cast copies at all.
  - qr tiles are [128, 2048] HALVES (h0 = HW cols 0..2047, h1 = rest),
    allocated h0-major, so batch-1's first-half loads recycle buffers
    ~30us earlier than whole-row tiles would allow; all consumers
    (transpose chunks, fp8 casts, mm2 residual reads) split cleanly at
    the 2048 boundary.
  - sim (mm1) is f32r: x transposed on the PE via identity matmuls (psA
    PSUM staging + qt copyback alternating ACT/DVE), symmetric upper
    block rows only, 5 mirror blocks PE-transposed back. Softmax: DVE
    min-reduce, ACT exp with Z accumulation (f32r).
  - mm2 runs in fp8e4m3 DoubleRow: one instruction contracts TWO 128-row
    k-tiles (512 output rows at ~1cyc/row), halving mm2 PE time vs f32r.
    attn^T fp8 pair tiles [128,2,C] come from PE transposes + cast
    copyback; q fp8 pair tiles [128,2,HW] are cast wave-by-wave on
    ACT/DVE in <=512-col chunks strictly interleaved with the qt
    copybacks (GpSimd casts work bit-exactly but are ~5x slower AND
    throttle the PE clock via the shared power budget — avoid).
    The attn tiles hold UNSCALED exp values (fp8 is scale-free but this
    avoids denormal flushing); the residual (+x) and gamma/Z_c row
    scale are applied EXACTLY in f32 by mm2's fused PSUM->SBUF copyback
    (scalar_tensor_tensor: out = pf*rzg + x on DVE) reading qr, so fp8
    error only touches gamma*attn@q (gamma=0.435): measured end-to-end
    rel L2 1.076e-2 against the jax reference (gate is 2e-2).
    Measured HW exec 147-158us across runs (the PE clock duty-cycles
    between full and half speed run-to-run; the f32r baseline measured
    194us under identical tracing). PE idle within its span is ~15us.
  - mm2 PSUM rotates through the pfeat pool AND the (dead between mm1
    consumers and the next batch) psim pool, 4 banks deep, so the PE
    runs ahead of the DVE residual adds instead of gating on them.
  - Loads and non-final stores ride the SP (sync) HWDGE queue (a store
    dma_start on ACT's in-order queue would wait for its staging data
    and head-of-line-block the next phase's copybacks); the final
    batch's stores alternate across both queues to drain the tail
    faster, fine-grained (1024/512 cols) at the very end.
  - PE latency bubbles are filled across batches: batch-1's first-wave
    transposes run during batch-0's softmax chain (loads prefetched a
    phase early via half-tile recycling), and batch-0's trailing mm2
    row-blocks (mi2-h1, mi3) run during batch-1's softmax chain.
  - attn^T fp8 tiles are PER-ROW-BLOCK [128,4,128] (transpose of
    p_t[mi] only), so mm2's first row block — and its stores — start as
    soon as mi=0's softmax chain completes rather than after all four.
"""
import sys

if "/opt/trn_rl_repo" not in sys.path:
    sys.path.insert(0, "/opt/trn_rl_repo")

import numpy as np

B, C, H, W = 16, 512, 64, 64
HW = H * W
NCORES = 8
NB = B // NCORES          # batches per core
P = 128
CB = C // P               # 4 channel blocks
KN = HW // P              # 32 contraction chunks for sim
HHW = HW // 2             # qr half length (2048)

_BUILD_CACHE = {}


def build_bass():
    import concourse.bacc as bacc
    import concourse.tile as tile
    from concourse import mybir
    from concourse.masks import make_identity

    f32 = mybir.dt.float32
    f32r = mybir.dt.float32r
    fp8 = mybir.dt.float8e4
    AX = mybir.AxisListType
    ALU = mybir.AluOpType
    ACTF = mybir.ActivationFunctionType
    DR = mybir.MatmulPerfMode.DoubleRow

    nc = bacc.Bacc()
    x_ext = nc.declare_dram_parameter("x", [NB, C, HW], f32, isOutput=False)
    g_ext = nc.declare_dram_parameter("gamma", [1], f32, isOutput=False)
    o_ext = nc.declare_dram_parameter("out", [NB, C, HW], f32, isOutput=True)

    _flip = [0]
    WAVES = [(0, 256), (256, 256), (512, 512),
             (1024, 1024), (2048, 1024), (3072, 1024)]

    with tile.TileContext(nc) as tc:
        with (
            tc.tile_pool(name="const", bufs=1) as const,
            tc.tile_pool(name="qr", bufs=14) as qrp,
            tc.tile_pool(name="q8", bufs=4) as q8p,
            tc.tile_pool(name="qt", bufs=8) as qtp,
            tc.tile_pool(name="pp", bufs=4) as pp,
            tc.tile_pool(name="pt8", bufs=4) as pt8p,
            tc.tile_pool(name="osb", bufs=3) as osb,
            tc.tile_pool(name="tri", bufs=2) as trip,
            tc.tile_pool(name="vec", bufs=8) as vec,
            tc.tile_pool(name="psA", bufs=2, space="PSUM") as psA,
            tc.tile_pool(name="psim", bufs=4, space="PSUM") as psimp,
            tc.tile_pool(name="pfeat", bufs=2, space="PSUM") as pfeat,
        ):
            def copyback(dst, src):
                """PSUM->SBUF copy (with dtype cast), alternating ACT/DVE."""
                if _flip[0] % 2 == 0:
                    nc.scalar.copy(dst, src)
                else:
                    nc.vector.tensor_copy(dst, src)
                _flip[0] += 1

            def alloc_qr(b):
                # h0-major: all mi first halves, then all second halves,
                # so batch b+1's first halves recycle the oldest buffers
                tiles = [[None, None] for _ in range(CB)]
                for h in range(2):
                    for mi in range(CB):
                        tiles[mi][h] = qrp.tile(
                            [P, HHW], f32r, tag="qr", name=f"qr{b}_{mi}_{h}"
                        )
                return tiles

            def qr_slice(st, mi, c0, c1):
                h = c0 // HHW
                assert c1 <= (h + 1) * HHW
                return st["qr"][mi][h][:, c0 - h * HHW:c1 - h * HHW]

            def load_wave(b, st, w0, wlen):
                for mi in range(CB):
                    nc.sync.dma_start(
                        out=qr_slice(st, mi, w0, w0 + wlen),
                        in_=x_ext[b, mi * P:(mi + 1) * P,
                                  w0:w0 + wlen].bitcast(f32r),
                    )

            # batch-0 first-wave loads go out first so DMA runs during the
            # fixed engine-startup preamble
            st0 = {"qr": alloc_qr(0)}
            load_wave(0, st0, 0, 256)

            ident_f = const.tile([P, P], f32)
            make_identity(nc, ident_f)
            ident_r = const.tile([P, P], f32r)
            nc.vector.tensor_copy(ident_r[:], ident_f[:])
            gamma_sb = const.tile([P, 1], f32)
            nc.sync.dma_start(out=gamma_sb[:], in_=g_ext[:].to_broadcast([P, 1]))

            # clock-warm dummy transposes while the first loads land
            warm = psA.tile([P, C], f32r, tag="psA", name="warmup")
            for i in range(12):
                nc.tensor.transpose(warm[:, :P], ident_r[:], ident_r[:])

            C0S = [min(mi * P, 2 * P) for mi in range(CB)]  # 0,128,256,256

            def transpose_group(st, b, kn, qt_tiles):
                """PE-transpose x chunk kn of all 4 mi into a qt tile."""
                pst = psA.tile([P, C], f32r, tag="psA")
                for ci in range(CB):
                    nc.tensor.transpose(
                        pst[:, ci * P:(ci + 1) * P],
                        qr_slice(st, ci, kn * P, (kn + 1) * P),
                        ident_r[:],
                    )
                qt = qtp.tile([P, C], f32r, tag="qt", name=f"qt{b}_{kn}")
                qt_tiles[kn] = qt
                copyback(qt[:], pst[:])

            def phase1(b, st, qt_tiles, skip_kn=()):
                """loads, fp8 casts (Pool), transposes to qT, sim matmuls."""
                st["q8"] = [q8p.tile([P, 2, HW], fp8, tag="q8",
                                     name=f"q8_{b}_{j}") for j in range(2)]
                st["psim"] = [psimp.tile([P, C], f32, tag="psim",
                                         name=f"psim{b}_{i}") for i in range(CB)]
                psim = st["psim"]

                def mm1(kn):
                    for mi in range(CB):
                        c0 = C0S[mi]
                        nc.tensor.matmul(
                            psim[mi][:, c0:],
                            qt_tiles[kn][:, mi * P:(mi + 1) * P],
                            qt_tiles[kn][:, c0:],
                            start=(kn == 0),
                            stop=(kn == KN - 1),
                        )

                # fp8 q casts go on ACT/DVE in <=512-col chunks, emitted
                # one per transpose-group on the engine the copyback did
                # NOT use, so they never burst-delay the qt copybacks
                cast_q = []

                def emit_cast():
                    if not cast_q:
                        return
                    mi, c0, c1 = cast_q.pop(0)
                    dst = st["q8"][mi // 2][:, mi % 2, c0:c1]
                    src = qr_slice(st, mi, c0, c1)
                    if _flip[0] % 2 == 0:
                        nc.scalar.copy(dst, src)
                    else:
                        nc.vector.tensor_copy(dst, src)
                    _flip[0] += 1

                pending = []
                ngroups = [0]
                for wi, (w0, wlen) in enumerate(WAVES):
                    if not st.get("preloaded", 0) > w0:
                        load_wave(b, st, w0, wlen)
                    # enqueue this wave's cast chunks up front: the slot
                    # cadence (~1us/group) trails the per-mi DMA arrivals,
                    # and draining 2-per-slot when backlogged empties the
                    # queue BEFORE the softmax chain needs ACT/DVE (a
                    # phase-end cast burst sat on sm's critical path)
                    for mi in range(CB):
                        for c0 in range(w0, w0 + wlen, 512):
                            cast_q.append((mi, c0, min(c0 + 512, w0 + wlen)))
                    for kq in range(wlen // P):
                        kn = w0 // P + kq
                        if kn not in skip_kn:
                            # the first few copybacks go on ACT: at phase
                            # entry the DVE is still draining the previous
                            # mm2's scale-adds, and psA only has 2 bufs
                            if ngroups[0] < 4:
                                _flip[0] += _flip[0] % 2
                            transpose_group(st, b, kn, qt_tiles)
                            ngroups[0] += 1
                        emit_cast()
                        if len(cast_q) > 8:
                            emit_cast()
                        pending.append(kn)
                        if len(pending) > 2:
                            mm1(pending.pop(0))
                for kn in pending:
                    mm1(kn)
                while cast_q:
                    emit_cast()
                return st

            def softmax_pt(b, st, filler=None):
                """rowwise softmax + tri fills, fp8 attn^T pair tiles."""
                psim = st["psim"]
                st["rzg"] = []
                st["pt8"] = []
                if filler is not None:
                    filler()  # PE work to hide the softmax chain latency

                def sm_mi(mi):
                    mrow = vec.tile([P, 1], f32, tag="mrow")
                    nc.vector.tensor_reduce(
                        mrow[:], psim[mi][:], axis=AX.X, op=ALU.min
                    )
                    zrow = vec.tile([P, 1], f32, tag="zrow")
                    # p stays UNSCALED (exp in [0,1] is fp8-friendly); the
                    # gamma/Z row scale is applied by mm2's fused
                    # scale-and-add, off the softmax critical path
                    p_t = pp.tile([P, C], f32r, tag="p", bufs=4)
                    nc.scalar.activation(
                        p_t[:], psim[mi][:], ACTF.Exp,
                        bias=mrow[:], scale=-1.0, accum_out=zrow[:],
                    )
                    rz = vec.tile([P, 1], f32, tag="rz")
                    nc.vector.reciprocal(rz[:], zrow[:])
                    rzg = vec.tile([P, 1], f32, tag="rzg", bufs=8)
                    nc.vector.tensor_mul(rzg[:], rz[:], gamma_sb[:])
                    st["rzg"].append(rzg)
                    pstm = pfeat.tile([P, C], f32r, tag="pf")
                    for kd in range(CB):
                        nc.tensor.transpose(
                            pstm[:, kd * P:(kd + 1) * P],
                            p_t[:, kd * P:(kd + 1) * P],
                            ident_r[:],
                        )
                    t8 = pt8p.tile([P, CB, P], fp8, tag="pt8",
                                   name=f"pt8_{b}_{mi}", bufs=8)
                    copyback(t8[:].rearrange("p f m -> p (f m)"), pstm[:])
                    st["pt8"].append(t8)

                # mi=0 needs no mirror fill: its chain (and mm2's first
                # row block) starts before the tri copies occupy ACT
                sm_mi(0)
                # mirror-block fills; deepest row (mi=3) first to unblock
                # its softmax chain earliest
                for (i, j) in [(3, 0), (3, 1), (1, 0), (2, 0), (2, 1)]:
                    tmp = trip.tile([P, P], f32, tag="tri")
                    nc.scalar.copy(tmp[:], psim[j][:, i * P:(i + 1) * P])
                    nc.tensor.transpose(
                        psim[i][:, j * P:(j + 1) * P], tmp[:], ident_f[:]
                    )
                for mi in range(1, CB):
                    sm_mi(mi)

            def mm2(b, st, mis=None, alt_psum=None):
                """feat = (gamma*attn) @ q in fp8 DoubleRow; +x on copyback.

                PSUM rotates through pfeat AND a second pool that is dead
                during this window (psim normally; psA when emitted as the
                softmax filler of the next batch) so the PE runs ahead of
                the DVE residual adds.
                """
                pt8, q8 = st["pt8"], st["q8"]
                last = (b == NB - 1)
                nalloc = [0]
                if mis is None:
                    parts = [(mi, h) for mi in range(CB) for h in range(2)]
                else:
                    parts = list(mis)
                for mi, half in parts:
                    if last and mi == CB - 1:
                        sgran = 512
                    elif last and mi == CB - 2:
                        sgran = 1024
                    else:
                        sgran = 2048
                    if True:
                        stg = osb.tile([P, HW // 2], f32, tag="ot",
                                       name=f"stg{b}_{mi}_{half}")
                        for njh in range(4):
                            nj = half * 4 + njh
                            nalloc[0] += 1
                            if nalloc[0] % 2 == 0:
                                pf = pfeat.tile([P, 512], f32, tag="pf")
                            elif alt_psum is None:
                                pf = psimp.tile([P, 512], f32, tag="psim")
                            else:
                                pf = alt_psum.tile([P, 512], f32, tag="psA")
                            for jp in range(2):
                                nc.tensor.matmul(
                                    pf[:],
                                    pt8[mi][:, 2 * jp:2 * jp + 2, :],
                                    q8[jp][:, :, nj * 512:(nj + 1) * 512],
                                    start=(jp == 0),
                                    stop=(jp == 1),
                                    perf_mode=DR,
                                )
                            # out = (gamma/Z_c) * feat + x in one DVE op;
                            # pt8 holds UNSCALED exp(min-sim) so the gamma/Z
                            # row scale never sits on the softmax critical
                            # path
                            nc.vector.scalar_tensor_tensor(
                                stg[:, njh * 512:(njh + 1) * 512],
                                pf[:],
                                st["rzg"][mi][:],
                                qr_slice(st, mi, nj * 512, (nj + 1) * 512),
                                op0=ALU.mult,
                                op1=ALU.add,
                            )
                            done = (njh + 1) * 512
                            if done % sgran == 0:
                                s0 = done - sgran
                                # non-final stores ride the SP queue: a
                                # store dma_start on ACT's in-order queue
                                # waits for its staging data and head-of-
                                # line-blocks the next phase's copybacks.
                                # The final batch alternates both queues
                                # to drain the tail faster.
                                eng = nc.sync
                                if last:
                                    _flip[0] += 1
                                    eng = nc.sync if _flip[0] % 2 else nc.scalar
                                eng.dma_start(
                                    out=o_ext[b, mi * P:(mi + 1) * P,
                                              half * 2048 + s0:half * 2048 + done],
                                    in_=stg[:, s0:done],
                                )

            # emission order: see module docstring
            st0["preloaded"] = 256
            qt0 = {}
            phase1(0, st0, qt0)
            # prefetch batch-1 first halves while batch-0 computes
            st1 = {"qr": alloc_qr(1), "preloaded": 2048}
            for (w0, wlen) in WAVES[:4]:
                load_wave(1, st1, w0, wlen)
            qt1 = {}
            softmax_pt(0, st0, filler=lambda: [
                transpose_group(st1, 1, kn, qt1) for kn in (0, 1)
            ])
            mm2(0, st0, mis=[(0, 0), (0, 1), (1, 0), (1, 1), (2, 0)])
            phase1(1, st1, qt1, skip_kn=(0, 1))
            # batch-0's trailing row blocks fill batch-1's first softmax
            # chain (per-mi pt8 lets mm2(1)-mi0 start right afterwards)
            softmax_pt(1, st1,
                       filler=lambda: mm2(0, st0,
                                          mis=[(2, 1), (3, 0), (3, 1)],
                                          alt_psum=psA))
            mm2(1, st1)

    nc.finalize()
    return nc


def get_bass():
    if "nc" not in _BUILD_CACHE:
        _BUILD_CACHE["nc"] = build_bass()
    return _BUILD_CACHE["nc"]


def make_in_maps(x, gamma):
    x = np.ascontiguousarray(np.asarray(x, dtype=np.float32)).reshape(B, C, HW)
    gamma = np.asarray(gamma, dtype=np.float32).reshape(1)
    return [
        {"x": x[i * NB:(i + 1) * NB], "gamma": gamma}
        for i in range(NCORES)
    ]


def run(x, gamma, trace=False, **trace_kwargs):
    from concourse.bass_utils import run_bass_kernel_spmd

    nc = get_bass()
    res = run_bass_kernel_spmd(
        nc, make_in_maps(x, gamma), core_ids=list(range(NCORES)),
        trace=trace, **trace_kwargs,
    )
    out = np.concatenate([res.results[i]["out"] for i in range(NCORES)], axis=0)
    return out.reshape(B, C, H, W), res


def kernel(x, gamma):
    out, _ = run(x, gamma, trace=False)
    return out


# revision 38
# speedup vs baseline: 1.0244x; 1.0244x over previous
"""Channel-attention (CAM) Trainium2 kernel — fp8 DoubleRow mm2 version.

Reference computation (per batch b of 16):
    q   = x[b].reshape(C, HW)                  # C=512, HW=4096
    sim = q @ q.T                              # [C, C], symmetric
    attn = softmax(max(sim) - sim, axis=-1)    # == exp(min_r - sim) / Z_r
    out[b] = gamma * attn @ q + x[b]

Sharding: data-parallel over batch across 8 NeuronCores (2 batches/core).
kernel() takes full inputs, shards internally, returns the full output.

Design notes (per-core, derived from HW trace analysis of the f32r
baseline — PE sustains ~1.8-2.3GHz, 1 cyc/row f32r, transposes 1.5x):
  - x is DMA'd straight into f32r qr tiles (f32<->f32r bitcast on both
    sides of the DMA: f32r is an fp32-storage PE read mode, and the DMA
    itself satisfies the BIR verifier's "rounded to f32r" producer rule).
    No DVE cast copies at all.
  - qr tiles are [128, 2048] HALVES (h0 = HW cols 0..2047, h1 = rest),
    allocated h0-major, so batch-1's first-half loads recycle buffers
    ~30us earlier than whole-row tiles would allow; all consumers
    (transpose chunks, fp8 casts, mm2 residual reads) split cleanly at
    the 2048 boundary.
  - sim (mm1) is f32r: x transposed on the PE via identity matmuls (psA
    PSUM staging + qt copyback alternating ACT/DVE), symmetric upper
    block rows only, 5 mirror blocks PE-transposed back. Softmax: DVE
    min-reduce, ACT exp with Z accumulation (f32r).
  - mm2 runs in fp8e4m3 DoubleRow: one instruction contracts TWO 128-row
    k-tiles (512 output rows at ~1cyc/row), halving mm2 PE time vs f32r.
    attn^T fp8 pair tiles [128,2,C] come from PE transposes + cast
    copyback; q fp8 pair tiles [128,2,HW] are cast wave-by-wave on
    ACT/DVE in <=512-col chunks strictly interleaved with the qt
    copybacks (GpSimd casts work bit-exactly but are ~5x slower AND
    throttle the PE clock via the shared power budget — avoid).
    The attn tiles hold UNSCALED exp values (fp8 is scale-free but this
    avoids denormal flushing); the residual (+x) and gamma/Z_c row
    scale are applied EXACTLY in f32 by mm2's fused PSUM->SBUF copyback
    (scalar_tensor_tensor: out = pf*rzg + x on DVE) reading qr, so fp8
    error only touches gamma*attn@q (gamma=0.435): measured end-to-end
    rel L2 1.076e-2 against the jax reference (gate is 2e-2).
    Measured HW exec 147-158us across runs (the PE clock duty-cycles
    between full and half speed run-to-run; the f32r baseline measured
    194us under identical tracing). PE idle within its span is ~15us.
  - mm2 PSUM rotates through the pfeat pool AND the (dead between mm1
    consumers and the next batch) psim pool, 4 banks deep, so the PE
    runs ahead of the DVE residual adds instead of gating on them.
  - Loads and non-final stores ride the SP (sync) HWDGE queue (a store
    dma_start on ACT's in-order queue would wait for its staging data
    and head-of-line-block the next phase's copybacks); the final
    batch's stores alternate across both queues to drain the tail
    faster, fine-grained (1024/512 cols) at the very end.
  - PE latency bubbles are filled across batches: batch-1's first-wave
    transposes run during batch-0's softmax chain (loads prefetched a
    phase early via half-tile recycling), and batch-0's trailing mm2
    row-blocks (mi2-h1, mi3) run during batch-1's softmax chain.
  - attn^T fp8 tiles are PER-ROW-BLOCK [128,4,128] (transpose of
    p_t[mi] only), so mm2's first row block — and its stores — start as
    soon as mi=0's softmax chain completes rather than after all four.
"""
import sys

if "/opt/trn_rl_repo" not in sys.path:
    sys.path.insert(0, "/opt/trn_rl_repo")

import numpy as np

B, C, H, W = 16, 512, 64, 64
HW = H * W
NCORES = 8
NB = B // NCORES          # batches per core
P = 128
CB = C // P               # 4 channel blocks
KN = HW // P              # 32 contraction chunks for sim
HHW = HW // 2             # qr half length (2048)

_BUILD_CACHE = {}


def build_bass():
    import concourse.bacc as bacc
    import concourse.tile as tile
    from concourse import mybir
    from concourse.masks import make_identity

    f32 = mybir.dt.float32
    f32r = mybir.dt.float32r
    fp8 = mybir.dt.float8e4
    AX = mybir.AxisListType
    ALU = mybir.AluOpType
    ACTF = mybir.ActivationFunctionType
    DR = mybir.MatmulPerfMode.DoubleRow

    nc = bacc.Bacc()
    x_ext = nc.declare_dram_parameter("x", [NB, C, HW], f32, isOutput=False)
    g_ext = nc.declare_dram_parameter("gamma", [1], f32, isOutput=False)
    o_ext = nc.declare_dram_parameter("out", [NB, C, HW], f32, isOutput=True)

    _flip = [0]
    WAVES = [(0, 256), (256, 256), (512, 512),
             (1024, 1024), (2048, 1024), (3072, 1024)]

    with tile.TileContext(nc) as tc:
        with (
            tc.tile_pool(name="const", bufs=1) as const,
            tc.tile_pool(name="qr", bufs=14) as qrp,
            tc.tile_pool(name="q8", bufs=4) as q8p,
            tc.tile_pool(name="qt", bufs=8) as qtp,
            tc.tile_pool(name="pp", bufs=4) as pp,
            tc.tile_pool(name="pt8", bufs=4) as pt8p,
            tc.tile_pool(name="osb", bufs=3) as osb,
            tc.tile_pool(name="tri", bufs=2) as trip,
            tc.tile_pool(name="vec", bufs=8) as vec,
            tc.tile_pool(name="psA", bufs=2, space="PSUM") as psA,
            tc.tile_pool(name="psim", bufs=4, space="PSUM") as psimp,
            tc.tile_pool(name="pfeat", bufs=2, space="PSUM") as pfeat,
        ):
            def copyback(dst, src):
                """PSUM->SBUF copy (with dtype cast), alternating ACT/DVE."""
                if _flip[0] % 2 == 0:
                    nc.scalar.copy(dst, src)
                else:
                    nc.vector.tensor_copy(dst, src)
                _flip[0] += 1

            def alloc_qr(b):
                # h0-major: all mi first halves, then all second halves,
                # so batch b+1's first halves recycle the oldest buffers
                tiles = [[None, None] for _ in range(CB)]
                for h in range(2):
                    for mi in range(CB):
                        tiles[mi][h] = qrp.tile(
                            [P, HHW], f32r, tag="qr", name=f"qr{b}_{mi}_{h}"
                        )
                return tiles

            def qr_slice(st, mi, c0, c1):
                h = c0 // HHW
                assert c1 <= (h + 1) * HHW
                return st["qr"][mi][h][:, c0 - h * HHW:c1 - h * HHW]

            def load_wave(b, st, w0, wlen):
                for mi in range(CB):
                    nc.sync.dma_start(
                        out=qr_slice(st, mi, w0, w0 + wlen),
                        in_=x_ext[b, mi * P:(mi + 1) * P,
                                  w0:w0 + wlen].bitcast(f32r),
                    )

            # batch-0 first-wave loads go out first so DMA runs during the
            # fixed engine-startup preamble
            st0 = {"qr": alloc_qr(0)}
            load_wave(0, st0, 0, 256)

            ident_f = const.tile([P, P], f32)
            make_identity(nc, ident_f)
            ident_r = const.tile([P, P], f32r)
            nc.vector.tensor_copy(ident_r[:], ident_f[:])
            gamma_sb = const.tile([P, 1], f32)
            nc.sync.dma_start(out=gamma_sb[:], in_=g_ext[:].to_broadcast([P, 1]))

            # clock-warm dummy transposes while the first loads land
            warm = psA.tile([P, C], f32r, tag="psA", name="warmup")
            for i in range(12):
                nc.tensor.transpose(warm[:, :P], ident_r[:], ident_r[:])

            C0S = [min(mi * P, 2 * P) for mi in range(CB)]  # 0,128,256,256

            def transpose_group(st, b, kn, qt_tiles, use_pf=False):
                """PE-transpose x chunk kn of all 4 mi into a qt tile."""
                if use_pf:
                    # deepen the staging ring with the (idle until the
                    # softmax phase) pfeat bank
                    pst = pfeat.tile([P, C], f32r, tag="pf")
                else:
                    pst = psA.tile([P, C], f32r, tag="psA")
                for ci in range(CB):
                    nc.tensor.transpose(
                        pst[:, ci * P:(ci + 1) * P],
                        qr_slice(st, ci, kn * P, (kn + 1) * P),
                        ident_r[:],
                    )
                qt = qtp.tile([P, C], f32r, tag="qt", name=f"qt{b}_{kn}")
                qt_tiles[kn] = qt
                copyback(qt[:], pst[:])

            def phase1(b, st, qt_tiles, skip_kn=()):
                """loads, fp8 casts (Pool), transposes to qT, sim matmuls."""
                st["q8"] = [q8p.tile([P, 2, HW], fp8, tag="q8",
                                     name=f"q8_{b}_{j}") for j in range(2)]
                st["psim"] = [psimp.tile([P, C], f32, tag="psim",
                                         name=f"psim{b}_{i}") for i in range(CB)]
                psim = st["psim"]

                def mm1(kn):
                    for mi in range(CB):
                        c0 = C0S[mi]
                        nc.tensor.matmul(
                            psim[mi][:, c0:],
                            qt_tiles[kn][:, mi * P:(mi + 1) * P],
                            qt_tiles[kn][:, c0:],
                            start=(kn == 0),
                            stop=(kn == KN - 1),
                        )

                # fp8 q casts go on ACT/DVE in <=512-col chunks, emitted
                # one per transpose-group on the engine the copyback did
                # NOT use, so they never burst-delay the qt copybacks
                cast_q = []

                def emit_cast():
                    if not cast_q:
                        return
                    mi, c0, c1 = cast_q.pop(0)
                    dst = st["q8"][mi // 2][:, mi % 2, c0:c1]
                    src = qr_slice(st, mi, c0, c1)
                    if _flip[0] % 2 == 0:
                        nc.scalar.copy(dst, src)
                    else:
                        nc.vector.tensor_copy(dst, src)
                    _flip[0] += 1

                pending = []
                ngroups = [0]
                for wi, (w0, wlen) in enumerate(WAVES):
                    if not st.get("preloaded", 0) > w0:
                        load_wave(b, st, w0, wlen)
                    # enqueue this wave's cast chunks up front: the slot
                    # cadence (~1us/group) trails the per-mi DMA arrivals,
                    # and draining 2-per-slot when backlogged empties the
                    # queue BEFORE the softmax chain needs ACT/DVE (a
                    # phase-end cast burst sat on sm's critical path)
                    for mi in range(CB):
                        for c0 in range(w0, w0 + wlen, 512):
                            cast_q.append((mi, c0, min(c0 + 512, w0 + wlen)))
                    for kq in range(wlen // P):
                        kn = w0 // P + kq
                        if kn not in skip_kn:
                            # the first few copybacks go on ACT: at phase
                            # entry the DVE is still draining the previous
                            # mm2's scale-adds, and psA only has 2 bufs
                            if ngroups[0] < 4:
                                _flip[0] += _flip[0] % 2
                            transpose_group(st, b, kn, qt_tiles,
                                            use_pf=(wi >= 3 and kn % 3 == 2))
                            ngroups[0] += 1
                        emit_cast()
                        if len(cast_q) > 8:
                            emit_cast()
                        pending.append(kn)
                        if len(pending) > 2:
                            mm1(pending.pop(0))
                for kn in pending:
                    mm1(kn)
                while cast_q:
                    emit_cast()
                return st

            def softmax_pt(b, st, filler=None):
                """rowwise softmax + tri fills, fp8 attn^T pair tiles."""
                psim = st["psim"]
                st["rzg"] = []
                st["pt8"] = []
                if filler is not None:
                    filler()  # PE work to hide the softmax chain latency

                def sm_mi(mi):
                    mrow = vec.tile([P, 1], f32, tag="mrow")
                    nc.vector.tensor_reduce(
                        mrow[:], psim[mi][:], axis=AX.X, op=ALU.min
                    )
                    zrow = vec.tile([P, 1], f32, tag="zrow")
                    # p stays UNSCALED (exp in [0,1] is fp8-friendly); the
                    # gamma/Z row scale is applied by mm2's fused
                    # scale-and-add, off the softmax critical path
                    p_t = pp.tile([P, C], f32r, tag="p", bufs=4)
                    nc.scalar.activation(
                        p_t[:], psim[mi][:], ACTF.Exp,
                        bias=mrow[:], scale=-1.0, accum_out=zrow[:],
                    )
                    rz = vec.tile([P, 1], f32, tag="rz")
                    nc.vector.reciprocal(rz[:], zrow[:])
                    rzg = vec.tile([P, 1], f32, tag="rzg", bufs=8)
                    nc.vector.tensor_mul(rzg[:], rz[:], gamma_sb[:])
                    st["rzg"].append(rzg)
                    pstm = pfeat.tile([P, C], f32r, tag="pf")
                    for kd in range(CB):
                        nc.tensor.transpose(
                            pstm[:, kd * P:(kd + 1) * P],
                            p_t[:, kd * P:(kd + 1) * P],
                            ident_r[:],
                        )
                    t8 = pt8p.tile([P, CB, P], fp8, tag="pt8",
                                   name=f"pt8_{b}_{mi}", bufs=8)
                    copyback(t8[:].rearrange("p f m -> p (f m)"), pstm[:])
                    st["pt8"].append(t8)

                # mi=0 needs no mirror fill: its chain (and mm2's first
                # row block) starts before the tri copies occupy ACT
                sm_mi(0)
                # mirror-block fills; deepest row (mi=3) first to unblock
                # its softmax chain earliest
                for (i, j) in [(3, 0), (3, 1), (1, 0), (2, 0), (2, 1)]:
                    tmp = trip.tile([P, P], f32, tag="tri")
                    nc.scalar.copy(tmp[:], psim[j][:, i * P:(i + 1) * P])
                    nc.tensor.transpose(
                        psim[i][:, j * P:(j + 1) * P], tmp[:], ident_f[:]
                    )
                for mi in range(1, CB):
                    sm_mi(mi)

            def mm2(b, st, mis=None, alt_psum=None):
                """feat = (gamma*attn) @ q in fp8 DoubleRow; +x on copyback.

                PSUM rotates through pfeat AND a second pool that is dead
                during this window (psim normally; psA when emitted as the
                softmax filler of the next batch) so the PE runs ahead of
                the DVE residual adds.
                """
                pt8, q8 = st["pt8"], st["q8"]
                last = (b == NB - 1)
                nalloc = [0]
                if mis is None:
                    parts = [(mi, h) for mi in range(CB) for h in range(2)]
                else:
                    parts = list(mis)
                for mi, half in parts:
                    if last and mi == CB - 1:
                        sgran = 512
                    elif last and mi == CB - 2:
                        sgran = 1024
                    else:
                        sgran = 2048
                    if True:
                        stg = osb.tile([P, HW // 2], f32, tag="ot",
                                       name=f"stg{b}_{mi}_{half}")
                        for njh in range(4):
                            nj = half * 4 + njh
                            nalloc[0] += 1
                            if nalloc[0] % 2 == 0:
                                pf = pfeat.tile([P, 512], f32, tag="pf")
                            elif alt_psum is None:
                                pf = psimp.tile([P, 512], f32, tag="psim")
                            else:
                                pf = alt_psum.tile([P, 512], f32, tag="psA")
                            for jp in range(2):
                                nc.tensor.matmul(
                                    pf[:],
                                    pt8[mi][:, 2 * jp:2 * jp + 2, :],
                                    q8[jp][:, :, nj * 512:(nj + 1) * 512],
                                    start=(jp == 0),
                                    stop=(jp == 1),
                                    perf_mode=DR,
                                )
                            # out = (gamma/Z_c) * feat + x in one DVE op;
                            # pt8 holds UNSCALED exp(min-sim) so the gamma/Z
                            # row scale never sits on the softmax critical
                            # path
                            nc.vector.scalar_tensor_tensor(
                                stg[:, njh * 512:(njh + 1) * 512],
                                pf[:],
                                st["rzg"][mi][:],
                                qr_slice(st, mi, nj * 512, (nj + 1) * 512),
                                op0=ALU.mult,
                                op1=ALU.add,
                            )
                            done = (njh + 1) * 512
                            if done % sgran == 0:
                                s0 = done - sgran
                                # non-final stores ride the SP queue: a
                                # store dma_start on ACT's in-order queue
                                # waits for its staging data and head-of-
                                # line-blocks the next phase's copybacks.
                                # The final batch alternates both queues
                                # to drain the tail faster.
                                eng = nc.sync
                                if last:
                                    _flip[0] += 1
                                    eng = nc.sync if _flip[0] % 2 else nc.scalar
                                eng.dma_start(
                                    out=o_ext[b, mi * P:(mi + 1) * P,
                                              half * 2048 + s0:half * 2048 + done],
                                    in_=stg[:, s0:done],
                                )

            # emission order: see module docstring
            st0["preloaded"] = 256
            qt0 = {}
            phase1(0, st0, qt0)
            # prefetch batch-1 first halves while batch-0 computes
            st1 = {"qr": alloc_qr(1), "preloaded": 2048}
            for (w0, wlen) in WAVES[:4]:
                load_wave(1, st1, w0, wlen)
            qt1 = {}
            softmax_pt(0, st0, filler=lambda: [
                transpose_group(st1, 1, kn, qt1) for kn in (0, 1)
            ])
            mm2(0, st0, mis=[(0, 0), (0, 1), (1, 0), (1, 1), (2, 0)])
            phase1(1, st1, qt1, skip_kn=(0, 1))
            # batch-0's trailing row blocks fill batch-1's first softmax
            # chain (per-mi pt8 lets mm2(1)-mi0 start right afterwards)
            softmax_pt(1, st1,
                       filler=lambda: mm2(0, st0,
                                          mis=[(2, 1), (3, 0), (3, 1)],
                                          alt_psum=psA))
            mm2(1, st1)

    nc.finalize()
    return nc


def get_bass():
    if "nc" not in _BUILD_CACHE:
        _BUILD_CACHE["nc"] = build_bass()
    return _BUILD_CACHE["nc"]


def make_in_maps(x, gamma):
    x = np.ascontiguousarray(np.asarray(x, dtype=np.float32)).reshape(B, C, HW)
    gamma = np.asarray(gamma, dtype=np.float32).reshape(1)
    return [
        {"x": x[i * NB:(i + 1) * NB], "gamma": gamma}
        for i in range(NCORES)
    ]


def run(x, gamma, trace=False, **trace_kwargs):
    from concourse.bass_utils import run_bass_kernel_spmd

    nc = get_bass()
    res = run_bass_kernel_spmd(
        nc, make_in_maps(x, gamma), core_ids=list(range(NCORES)),
        trace=trace, **trace_kwargs,
    )
    out = np.concatenate([res.results[i]["out"] for i in range(NCORES)], axis=0)
    return out.reshape(B, C, H, W), res


def kernel(x, gamma):
    out, _ = run(x, gamma, trace=False)
    return out
